# revision 1
# baseline (speedup 1.0000x reference)
"""GA3 Conv2d kernel for 8 Trainium2 NeuronCores.

Math: the reference computes, per batch image,
    out[b, co, m] = sum_{j,k} S[m,j,k] * (conv2d(a_k, W[j]) + bias[j])[co]
with a_k = x[:, k::8] (blade-interleaved channels).  Because the sign
combination is linear, it folds into the conv weights:
    V[co*8+m, ci*8+k, kh, kw] = sum_j S[m,j,k] * W[j, co, ci, kh, kw]
    bias_eff[co*8+m]          = sum_{j,k} S[m,j,k] * b[j, co]
so the whole module is ONE dense 3x3 conv with Cin=Cout=128 on
[B, 128, 128, 128].  We shard data-parallel over B across the 8 cores
(1 image per core) and implement the conv as 9 shifted fp16 matmuls per
4-row output block (tap weights stationary [ic=128 x oc=128], pixels
moving, fp32 PSUM accumulation; fp16 operands keep LDWEIGHTS fully
hidden behind the 512-column matmul stream and halve input DMA bytes;
measured rel err ~3e-4).

Layout: the host pre-pads each image into a flat per-partition buffer
    [pad pair][row: 128 data + pad pair] x 130 padded rows
(pitch 130, zeros at the halo) so every device-side load is a fully
contiguous DMA and tap shifts are pure address offsets.  All loads ride
the SP HWDGE ring in FIFO order (completion semaphores land right after
their own transfer); stores get the ACT ring.  Dep-free warm-up matmuls
on a memset scratch tile run during the head DMAs so the PE HAM clock
gate is already at 2.4 GHz when the data-gated real matmuls start.
"""

import numpy as np

_TERMS = [
    [(0, 0, 1), (1, 1, 1), (2, 2, 1), (3, 3, 1), (4, 4, -1), (5, 5, -1), (6, 6, -1), (7, 7, -1)],
    [(1, 0, 1), (0, 1, 1), (2, 4, 1), (4, 2, -1), (3, 6, 1), (6, 3, -1), (5, 7, -1), (7, 5, -1)],
    [(2, 0, 1), (0, 2, 1), (1, 4, -1), (4, 1, 1), (3, 5, 1), (5, 3, -1), (6, 7, 1), (7, 6, 1)],
    [(3, 0, 1), (0, 3, 1), (1, 6, -1), (6, 1, 1), (2, 5, -1), (5, 2, 1), (4, 7, -1), (7, 4, -1)],
    [(4, 0, 1), (0, 4, 1), (2, 1, 1), (1, 2, -1), (3, 7, 1), (7, 3, 1), (6, 5, 1), (5, 6, -1)],
    [(5, 0, 1), (0, 5, 1), (3, 2, 1), (2, 3, -1), (1, 7, 1), (7, 1, 1), (4, 6, 1), (6, 4, -1)],
    [(6, 0, 1), (0, 6, 1), (3, 1, 1), (1, 3, -1), (2, 7, -1), (7, 2, -1), (5, 4, 1), (4, 5, -1)],
    [(7, 0, 1), (0, 7, 1), (5, 1, 1), (1, 5, 1), (6, 2, -1), (2, 6, -1), (4, 3, 1), (3, 4, 1)],
]
_S = np.zeros((8, 8, 8), dtype=np.float32)
for _m, _terms in enumerate(_TERMS):
    for _j, _k, _s in _terms:
        _S[_m, _j, _k] = _s

B, CIN, COUT, H, W = 8, 16, 16, 128, 128
C = 8 * CIN  # 128 interleaved channels
N_CORES = 8
STRIP = 16          # output rows per strip (one store DMA)
N_STRIPS = H // STRIP
GROUP = 4           # output rows per PSUM accumulation group (4*128 = 512 free)
PW = W + 2          # padded row pitch in the flat layout
NPR = H + 2         # padded rows (-1 .. 128)
FLAT = 2 + NPR * PW  # flat elems/partition: leading pad pair + 130 rows
WCOLS = 9 * C + 1   # packed weight taps + bias column
N_WARMUP = 11       # HAM warm-up matmuls during the head DMAs

_CACHED_NC = None


def _build_nc():
    import concourse.bass as bass
    import concourse.mybir as mybir
    import concourse.tile as tile
    from concourse import bacc

    f32 = mybir.dt.float32
    f16 = mybir.dt.float16

    nc = bacc.Bacc("TRN2", target_bir_lowering=False, debug=False,
                   enable_asserts=False)

    xb = nc.dram_tensor("xb", [C, FLAT], f16, kind="ExternalInput").ap()
    wf = nc.dram_tensor("wf", [C, WCOLS], f16, kind="ExternalInput").ap()
    out = nc.dram_tensor("out", [C, H, W], f32, kind="ExternalOutput").ap()

    with tile.TileContext(nc) as tc:
        with (
            tc.tile_pool(name="wpool", bufs=1) as wpool,
            tc.tile_pool(name="xpool", bufs=1) as xpool,
            tc.tile_pool(name="pspool", bufs=8, space="PSUM") as pspool,
            tc.tile_pool(name="opool", bufs=3) as opool,
        ):
            xfull = xpool.tile([C, FLAT], f16)
            wtile = wpool.tile([C, WCOLS], f16)

            # All loads ride the SP ring in FIFO order (wf first), so each
            # DMA's completion semaphore lands right after its own transfer
            # instead of draining behind cross-ring traffic.  Stores get the
            # ACT ring to themselves.
            nc.sync.dma_start(out=wtile[:, :], in_=wf[:, :])
            # DVE tensor_scalar needs an fp32 scalar operand — up-convert the
            # packed fp16 bias column once
            btile = wpool.tile([C, 1], f32)
            nc.vector.tensor_copy(out=btile[:, :], in_=wtile[:, 9 * C:WCOLS])

            # HAM warm-up: dep-free junk matmuls on a memset scratch tile run
            # right after the preamble, lifting the PE clock gate to 2.4 GHz
            # before the data-gated real matmuls begin.
            wmsrc = wpool.tile([C, 512], f16)
            nc.vector.memset(wmsrc[:, :], 0.0)
            wmps = pspool.tile([C, GROUP * W], f32, tag="ps")
            for _ in range(N_WARMUP):
                nc.tensor.matmul(wmps[:, :], lhsT=wmsrc[:, 0:C],
                                 rhs=wmsrc[:, 0:512], start=True, stop=True,
                                 skip_group_check=True)

            # input chunks (contiguous flat ranges at padded-row boundaries);
            # first chunk halved so the first PSUM group starts early.  Only
            # the first three are issued at the head — the rest are emitted
            # interleaved with the strips so head DMAs don't collide on the
            # 8 shared DMA-completion semaphore lanes.
            bounds = [0, 6, 9, 25, 41, 57, 73, 89, 105, 121, NPR]

            def emit_chunk(c):
                a, b = bounds[c], bounds[c + 1]
                lo = 0 if a == 0 else 2 + PW * a
                hi = 2 + PW * b if b < NPR else FLAT
                nc.sync.dma_start(out=xfull[:, lo:hi], in_=xb[:, lo:hi])

            for c in range(4):
                emit_chunk(c)

            # ---- conv: 32 PSUM groups x 9 shifted matmuls
            for s in range(N_STRIPS):
                if 1 <= s <= 6:
                    emit_chunk(s + 3)   # stays ~2 strips ahead of consumption
                obuf = opool.tile([C, STRIP * W], f32)
                for g in range(STRIP // GROUP):
                    ps = pspool.tile([C, GROUP * W], f32)
                    ta = 0
                    for dh in range(3):
                        for dw in range(3):
                            # out rows R..R+3 read padded rows R+dh..R+dh+3
                            # at column shift dw-1; padded row pr's data
                            # starts at flat offset 2 + 130*pr
                            base = 1 + PW * (16 * s + 4 * g + dh) + dw
                            rhs = bass.AP(xfull.tensor, xfull.offset + base,
                                          [xfull.ap[0], [PW, GROUP], [1, W]])
                            nc.tensor.matmul(
                                ps[:, :],
                                lhsT=wtile[:, ta * C:(ta + 1) * C],
                                rhs=rhs,
                                start=(ta == 0),
                                stop=(ta == 8),
                            )
                            ta += 1
                    nc.vector.tensor_scalar_add(
                        out=obuf[:, g * GROUP * W:(g + 1) * GROUP * W],
                        in0=ps[:, :],
                        scalar1=btile[:, 0:1],
                    )
                if s < N_STRIPS - 1:
                    nc.scalar.dma_start(
                        out=out[:, s * STRIP:(s + 1) * STRIP, :],
                        in_=obuf[:, :])
                else:
                    # split the final store so the tail barrier waits on a
                    # small last transfer
                    for g in range(STRIP // GROUP):
                        nc.scalar.dma_start(
                            out=out[:, s * STRIP + g * GROUP:
                                    s * STRIP + (g + 1) * GROUP, :],
                            in_=obuf[:, g * GROUP * W:(g + 1) * GROUP * W])

    nc.compile()
    return nc


def _get_nc():
    global _CACHED_NC
    if _CACHED_NC is None:
        _CACHED_NC = _build_nc()
    return _CACHED_NC


def _prep_weights(Wfull: np.ndarray, b: np.ndarray):
    # wf[ic, tap*128 + oc] with ic = ci*8+k, oc = co*8+m, tap = kh*3+kw;
    # final column (index 9*128) holds bias_eff[oc] indexed by partition.
    V = np.einsum("mjk,jcihw->ikhwcm", _S.astype(np.float64),
                  Wfull.astype(np.float64))          # [ci,k,kh,kw,co,m]
    V = V.reshape(C, 9 * C)
    bias = np.einsum("mjk,jc->cm", _S.astype(np.float64),
                     b.astype(np.float64)).reshape(C, 1)
    wf = np.concatenate([V, bias], axis=1)
    return np.ascontiguousarray(wf, dtype=np.float16)


def _pad_images(x: np.ndarray) -> np.ndarray:
    # [B, C, H, W] -> flat padded [B, C, FLAT] (see module docstring)
    xpad = np.zeros((x.shape[0], C, FLAT), dtype=np.float16)
    arr = xpad[:, :, 2:].reshape(x.shape[0], C, NPR, PW)
    arr[:, :, 1:H + 1, 0:W] = x
    return xpad


def kernel(x: np.ndarray, W: np.ndarray, b: np.ndarray) -> np.ndarray:
    from concourse.bass_utils import run_bass_kernel_spmd

    xpad = _pad_images(np.ascontiguousarray(x, dtype=np.float32))
    wf = _prep_weights(np.asarray(W), np.asarray(b))

    nc = _get_nc()
    in_maps = [{"xb": xpad[c], "wf": wf} for c in range(N_CORES)]
    res = run_bass_kernel_spmd(nc, in_maps, core_ids=list(range(N_CORES)))
    return np.stack([res.results[c]["out"] for c in range(N_CORES)], axis=0)



# revision 13
# speedup vs baseline: 1.0295x; 1.0295x over previous
"""GA3 Conv2d kernel for 8 Trainium2 NeuronCores — Winograd F(2,3) along H.

Math: the sign-combination einsum folds into the conv weights, making the
module ONE dense 3x3 conv with Cin=Cout=128 on [B, 128, 128, 128] (see
_prep math below).  We shard data-parallel over B (1 image per core).

The direct 9-tap matmul formulation is PE-bound: 9 taps x 512 cols x 32
groups = 147k PE cycles, ~70 us at the power-throttled ~2.1 GHz clock.
Winograd F(2,3) applied along H cuts PE work to 2/3: for each pair of
output rows (2c, 2c+1) and input rows d0..d3 = padded rows 2c..2c+3,
    m0 = conv_w(t0) @ G0,  t0 = d0 - d2,   G0 = W[dh=0]
    m1 = conv_w(t1) @ G1,  t1 = d1 + d2,   G1 = (W0+W1+W2)/2
    m2 = conv_w(t2) @ G2,  t2 = d2 - d1,   G2 = (W0-W1+W2)/2
    m3 = conv_w(t3) @ G3,  t3 = d1 - d3,   G3 = W[dh=2]
    even row = m0 + m1 + m2 + bias ;  odd row = m1 - m2 - m3 + bias
where conv_w is the remaining direct 3-tap conv along W (dw = 0..2).
The row transforms t0..t3 are LINEAR and cheap -> computed on HOST and
shipped as fp16 planes (2x input bytes, still under the PE roofline).
The output combine runs on DVE (even rows) and GPSIMD (odd rows) as two
fused scalar_tensor_tensor passes each, overlapping the PE stream; the
bias rides the STT per-partition scalar operand for free.

Layout: per tile c (c = 0..63) the host packs 4 width-padded rows
[t0 | t3 | t1 | t2], each [1 pad][128 data][1 pad] = 130 elems, so every
matmul rhs is a [520-stride x 4 tiles][1 x 128] AP and all loads are
contiguous chunks.  Even/odd output rows go to separate DRAM planes in
fp16; the host interleaves and upcasts (measured rel err ~5e-4).
"""

import numpy as np

_TERMS = [
    [(0, 0, 1), (1, 1, 1), (2, 2, 1), (3, 3, 1), (4, 4, -1), (5, 5, -1), (6, 6, -1), (7, 7, -1)],
    [(1, 0, 1), (0, 1, 1), (2, 4, 1), (4, 2, -1), (3, 6, 1), (6, 3, -1), (5, 7, -1), (7, 5, -1)],
    [(2, 0, 1), (0, 2, 1), (1, 4, -1), (4, 1, 1), (3, 5, 1), (5, 3, -1), (6, 7, 1), (7, 6, 1)],
    [(3, 0, 1), (0, 3, 1), (1, 6, -1), (6, 1, 1), (2, 5, -1), (5, 2, 1), (4, 7, -1), (7, 4, -1)],
    [(4, 0, 1), (0, 4, 1), (2, 1, 1), (1, 2, -1), (3, 7, 1), (7, 3, 1), (6, 5, 1), (5, 6, -1)],
    [(5, 0, 1), (0, 5, 1), (3, 2, 1), (2, 3, -1), (1, 7, 1), (7, 1, 1), (4, 6, 1), (6, 4, -1)],
    [(6, 0, 1), (0, 6, 1), (3, 1, 1), (1, 3, -1), (2, 7, -1), (7, 2, -1), (5, 4, 1), (4, 5, -1)],
    [(7, 0, 1), (0, 7, 1), (5, 1, 1), (1, 5, 1), (6, 2, -1), (2, 6, -1), (4, 3, 1), (3, 4, 1)],
]
_S = np.zeros((8, 8, 8), dtype=np.float32)
for _m, _terms in enumerate(_TERMS):
    for _j, _k, _s in _terms:
        _S[_m, _j, _k] = _s

B, CIN, COUT, H, W = 8, 16, 16, 128, 128
C = 8 * CIN          # 128 interleaved channels
N_CORES = 8
NT = H // 2          # 64 row-tiles (one per output row pair)
PWR = W + 2          # padded row: [pad][128][pad]
TBLK = 4 * PWR       # per-tile block: rows [t0, t3, t1, t2]
FLAT = NT * TBLK     # flat elems/partition
ROWOFF = [0, 2 * PWR, 3 * PWR, PWR]   # plane offset of t_u within a block
GPT = 4              # tiles per PSUM group (4 tiles = 4 even + 4 odd rows)
NG = NT // GPT       # 16 groups
FD = GPT * W         # 512 matmul free dim / combine span
WCOLS = 12 * C + 1   # 12 transformed weight mats + bias column
N_WARMUP = 11        # HAM warm-up matmuls during the head DMAs

_CACHED_NC = None


def _build_nc():
    import concourse.bass as bass
    import concourse.mybir as mybir
    import concourse.tile as tile
    from concourse import bacc

    f32 = mybir.dt.float32
    f16 = mybir.dt.float16
    ADD = mybir.AluOpType.add
    SUB = mybir.AluOpType.subtract
    MUL = mybir.AluOpType.mult

    nc = bacc.Bacc("TRN2", target_bir_lowering=False, debug=False,
                   enable_asserts=False)

    xb = nc.dram_tensor("xb", [C, FLAT], f16, kind="ExternalInput").ap()
    wf = nc.dram_tensor("wf", [C, WCOLS], f16, kind="ExternalInput").ap()
    oute = nc.dram_tensor("oute", [C, NT, W], f16, kind="ExternalOutput").ap()
    outo = nc.dram_tensor("outo", [C, NT, W], f16, kind="ExternalOutput").ap()

    with tile.TileContext(nc) as tc:
        with (
            tc.tile_pool(name="wpool", bufs=1) as wpool,
            tc.tile_pool(name="xpool", bufs=1) as xpool,
            tc.tile_pool(name="pspool", bufs=2, space="PSUM") as pspool,
            tc.tile_pool(name="tpool", bufs=8) as tpool,
            tc.tile_pool(name="opool", bufs=6) as opool,
        ):
            xfull = xpool.tile([C, FLAT], f16)
            wtile = wpool.tile([C, WCOLS], f16)

            # Loads ride the SP ring in FIFO order; stores get the ACT ring.
            nc.sync.dma_start(out=wtile[:, :], in_=wf[:, :])
            btile = wpool.tile([C, 1], f32)
            nc.vector.tensor_copy(out=btile[:, :], in_=wtile[:, 12 * C:WCOLS])

            # HAM warm-up: dep-free junk matmuls lift the PE clock gate
            # while the head DMAs stream in.
            wmsrc = wpool.tile([C, 512], f16)
            nc.vector.memset(wmsrc[:, :], 0.0)
            wmps = pspool.tile([C, FD], f32, tag="ps0")
            for _ in range(N_WARMUP):
                nc.tensor.matmul(wmps[:, :], lhsT=wmsrc[:, 0:C],
                                 rhs=wmsrc[:, 0:512], start=True, stop=True,
                                 skip_group_check=True)

            # input chunks: one group's 4 tile-blocks each (contiguous)
            def emit_chunk(g):
                lo, hi = g * GPT * TBLK, (g + 1) * GPT * TBLK
                nc.sync.dma_start(out=xfull[:, lo:hi], in_=xb[:, lo:hi])

            for g in range(4):
                emit_chunk(g)

            for g in range(NG):
                if g < NG - 4:
                    emit_chunk(g + 4)
                ps = [pspool.tile([C, FD], f32, name=f"psu{u}", tag=f"ps{u}")
                      for u in range(4)]
                for u in range(4):
                    for dw in range(3):
                        base = g * GPT * TBLK + ROWOFF[u] + dw
                        rhs = bass.AP(xfull.tensor, xfull.offset + base,
                                      [xfull.ap[0], [TBLK, GPT], [1, W]])
                        nc.tensor.matmul(
                            ps[u][:, :],
                            lhsT=wtile[:, (u * 3 + dw) * C:(u * 3 + dw + 1) * C],
                            rhs=rhs,
                            start=(dw == 0),
                            stop=(dw == 2),
                        )
                # Engine constraints: GPSIMD cannot touch PSUM; DVE may read
                # only ONE PSUM operand per op; ACT is single-tensor but
                # reads PSUM and applies a per-partition bias for free.
                # ACT: s1 = m1 + bias, s2 = m2   (PSUM -> SBUF)
                s1 = tpool.tile([C, FD], f32)
                s2 = tpool.tile([C, FD], f32)
                nc.scalar.add(out=s1[:, :], in_=ps[1][:, :], add=btile[:, 0:1])
                nc.scalar.copy(out=s2[:, :], in_=ps[2][:, :])
                # DVE: tmp_e = m0 + s1 ;  GPSIMD: even = tmp_e + s2
                tmp_e = tpool.tile([C, FD], f32)
                obuf_e = opool.tile([C, FD], f16)
                nc.vector.scalar_tensor_tensor(
                    out=tmp_e[:, :], in0=ps[0][:, :], scalar=0.0,
                    in1=s1[:, :], op0=ADD, op1=ADD)
                nc.gpsimd.tensor_add(obuf_e[:, :], tmp_e[:, :], s2[:, :])
                # GPSIMD: tmp_o = s1 - s2 ;  DVE: odd = tmp_o - m3
                tmp_o = tpool.tile([C, FD], f32)
                obuf_o = opool.tile([C, FD], f16)
                nc.gpsimd.tensor_sub(tmp_o[:, :], s1[:, :], s2[:, :])
                nc.vector.scalar_tensor_tensor(
                    out=obuf_o[:, :], in0=ps[3][:, :], scalar=-1.0,
                    in1=tmp_o[:, :], op0=MUL, op1=ADD)
                nc.scalar.dma_start(out=oute[:, g * GPT:(g + 1) * GPT, :],
                                    in_=obuf_e[:, :])
                nc.scalar.dma_start(out=outo[:, g * GPT:(g + 1) * GPT, :],
                                    in_=obuf_o[:, :])

    nc.compile()
    return nc


def _get_nc():
    global _CACHED_NC
    if _CACHED_NC is None:
        _CACHED_NC = _build_nc()
    return _CACHED_NC


def _prep_weights(Wfull: np.ndarray, b: np.ndarray):
    # V[ci*8+k, dh, dw, co*8+m] = sum_j S[m,j,k] * W[j, co, ci, dh, dw]
    V = np.einsum("mjk,jcihw->ikhwcm", _S.astype(np.float64),
                  np.asarray(Wfull).astype(np.float64)).reshape(C, 3, 3, C)
    G = [V[:, 0], (V[:, 0] + V[:, 1] + V[:, 2]) / 2,
         (V[:, 0] - V[:, 1] + V[:, 2]) / 2, V[:, 2]]   # each [ic, dw, oc]
    wf = np.empty((C, WCOLS), dtype=np.float16)
    for u in range(4):
        for dw in range(3):
            wf[:, (u * 3 + dw) * C:(u * 3 + dw + 1) * C] = G[u][:, dw, :]
    bias = np.einsum("mjk,jc->cm", _S.astype(np.float64),
                     np.asarray(b).astype(np.float64)).reshape(C)
    wf[:, 12 * C] = bias.astype(np.float16)
    return np.ascontiguousarray(wf)


def _prep_inputs(x: np.ndarray) -> np.ndarray:
    # [B, C, H, W] -> Winograd row-transformed flat planes [B, C, FLAT]
    nB = x.shape[0]
    pr = np.zeros((nB, C, H + 2, W), dtype=np.float32)
    pr[:, :, 1:-1, :] = x
    xt = np.zeros((nB, C, NT, 4, PWR), dtype=np.float16)
    xt[:, :, :, 0, 1:W + 1] = pr[:, :, 0:2 * NT:2] - pr[:, :, 2:2 * NT + 2:2]
    xt[:, :, :, 1, 1:W + 1] = pr[:, :, 1:2 * NT + 1:2] - pr[:, :, 3:2 * NT + 3:2]
    xt[:, :, :, 2, 1:W + 1] = pr[:, :, 1:2 * NT + 1:2] + pr[:, :, 2:2 * NT + 2:2]
    xt[:, :, :, 3, 1:W + 1] = pr[:, :, 2:2 * NT + 2:2] - pr[:, :, 1:2 * NT + 1:2]
    return xt.reshape(nB, C, FLAT)


def kernel(x: np.ndarray, W: np.ndarray, b: np.ndarray) -> np.ndarray:
    from concourse.bass_utils import run_bass_kernel_spmd

    xt = _prep_inputs(np.ascontiguousarray(x, dtype=np.float32))
    wf = _prep_weights(W, b)

    nc = _get_nc()
    in_maps = [{"xb": xt[c], "wf": wf} for c in range(N_CORES)]
    res = run_bass_kernel_spmd(nc, in_maps, core_ids=list(range(N_CORES)))
    out = np.empty((N_CORES, C, H, 128), dtype=np.float32)
    for c in range(N_CORES):
        out[c, :, 0::2, :] = res.results[c]["oute"].astype(np.float32)
        out[c, :, 1::2, :] = res.results[c]["outo"].astype(np.float32)
    return out


# revision 15
# speedup vs baseline: 1.0982x; 1.0666x over previous
"""GA3 Conv2d kernel for 8 Trainium2 NeuronCores — Winograd F(2,3) along H.

Math: the sign-combination einsum folds into the conv weights, making the
module ONE dense 3x3 conv with Cin=Cout=128 on [B, 128, 128, 128] (see
_prep math below).  We shard data-parallel over B (1 image per core).

The direct 9-tap matmul formulation is PE-bound: 9 taps x 512 cols x 32
groups = 147k PE cycles, ~70 us at the power-throttled ~2.1 GHz clock.
Winograd F(2,3) applied along H cuts PE work to 2/3: for each pair of
output rows (2c, 2c+1) and input rows d0..d3 = padded rows 2c..2c+3,
    m0 = conv_w(t0) @ G0,  t0 = d0 - d2,   G0 = W[dh=0]
    m1 = conv_w(t1) @ G1,  t1 = d1 + d2,   G1 = (W0+W1+W2)/2
    m2 = conv_w(t2) @ G2,  t2 = d2 - d1,   G2 = (W0-W1+W2)/2
    m3 = conv_w(t3) @ G3,  t3 = d1 - d3,   G3 = W[dh=2]
    even row = m0 + m1 + m2 + bias ;  odd row = m1 - m2 - m3 + bias
where conv_w is the remaining direct 3-tap conv along W (dw = 0..2).
The row transforms t0..t3 are LINEAR and cheap -> computed on HOST and
shipped as fp16 planes (2x input bytes, still under the PE roofline).
The output combine runs on DVE (even rows) and GPSIMD (odd rows) as two
fused scalar_tensor_tensor passes each, overlapping the PE stream; the
bias rides the STT per-partition scalar operand for free.

Layout: per tile c (c = 0..63) the host packs 4 width-padded rows
[t0 | t3 | t1 | t2], each [1 pad][128 data][1 pad] = 130 elems, so every
matmul rhs is a [520-stride x 4 tiles][1 x 128] AP and all loads are
contiguous chunks.  Even/odd output rows go to separate DRAM planes in
fp16; the host interleaves and upcasts (measured rel err ~5e-4).
"""

import numpy as np

_TERMS = [
    [(0, 0, 1), (1, 1, 1), (2, 2, 1), (3, 3, 1), (4, 4, -1), (5, 5, -1), (6, 6, -1), (7, 7, -1)],
    [(1, 0, 1), (0, 1, 1), (2, 4, 1), (4, 2, -1), (3, 6, 1), (6, 3, -1), (5, 7, -1), (7, 5, -1)],
    [(2, 0, 1), (0, 2, 1), (1, 4, -1), (4, 1, 1), (3, 5, 1), (5, 3, -1), (6, 7, 1), (7, 6, 1)],
    [(3, 0, 1), (0, 3, 1), (1, 6, -1), (6, 1, 1), (2, 5, -1), (5, 2, 1), (4, 7, -1), (7, 4, -1)],
    [(4, 0, 1), (0, 4, 1), (2, 1, 1), (1, 2, -1), (3, 7, 1), (7, 3, 1), (6, 5, 1), (5, 6, -1)],
    [(5, 0, 1), (0, 5, 1), (3, 2, 1), (2, 3, -1), (1, 7, 1), (7, 1, 1), (4, 6, 1), (6, 4, -1)],
    [(6, 0, 1), (0, 6, 1), (3, 1, 1), (1, 3, -1), (2, 7, -1), (7, 2, -1), (5, 4, 1), (4, 5, -1)],
    [(7, 0, 1), (0, 7, 1), (5, 1, 1), (1, 5, 1), (6, 2, -1), (2, 6, -1), (4, 3, 1), (3, 4, 1)],
]
_S = np.zeros((8, 8, 8), dtype=np.float32)
for _m, _terms in enumerate(_TERMS):
    for _j, _k, _s in _terms:
        _S[_m, _j, _k] = _s

B, CIN, COUT, H, W = 8, 16, 16, 128, 128
C = 8 * CIN          # 128 interleaved channels
N_CORES = 8
NT = H // 2          # 64 row-tiles (one per output row pair)
PWR = W + 2          # padded row: [pad][128][pad]
TBLK = 4 * PWR       # per-tile block: rows [t0, t3, t1, t2]
FLAT = NT * TBLK     # flat elems/partition
ROWOFF = [0, 2 * PWR, 3 * PWR, PWR]   # plane offset of t_u within a block
GPT = 4              # tiles per PSUM group (4 tiles = 4 even + 4 odd rows)
NG = NT // GPT       # 16 groups
FD = GPT * W         # 512 matmul free dim / combine span
WCOLS = 12 * C + 1   # 12 transformed weight mats + bias column
N_WARMUP = 11        # HAM warm-up matmuls during the head DMAs

_CACHED_NC = None


def _build_nc():
    import concourse.bass as bass
    import concourse.mybir as mybir
    import concourse.tile as tile
    from concourse import bacc

    f32 = mybir.dt.float32
    f16 = mybir.dt.float16
    ADD = mybir.AluOpType.add
    SUB = mybir.AluOpType.subtract
    MUL = mybir.AluOpType.mult

    nc = bacc.Bacc("TRN2", target_bir_lowering=False, debug=False,
                   enable_asserts=False)

    xb = nc.dram_tensor("xb", [C, FLAT], f16, kind="ExternalInput").ap()
    wf = nc.dram_tensor("wf", [C, WCOLS], f16, kind="ExternalInput").ap()
    oute = nc.dram_tensor("oute", [C, NT, W], f16, kind="ExternalOutput").ap()
    outo = nc.dram_tensor("outo", [C, NT, W], f16, kind="ExternalOutput").ap()

    with tile.TileContext(nc) as tc:
        with (
            tc.tile_pool(name="wpool", bufs=1) as wpool,
            tc.tile_pool(name="xpool", bufs=1) as xpool,
            tc.tile_pool(name="pspool", bufs=2, space="PSUM") as pspool,
            tc.tile_pool(name="tpool", bufs=8) as tpool,
            tc.tile_pool(name="opool", bufs=6) as opool,
        ):
            xfull = xpool.tile([C, FLAT], f16)
            wtile = wpool.tile([C, WCOLS], f16)

            # Loads ride the SP ring in FIFO order; stores get the ACT ring.
            nc.sync.dma_start(out=wtile[:, :], in_=wf[:, :])
            btile = wpool.tile([C, 1], f32)
            nc.vector.tensor_copy(out=btile[:, :], in_=wtile[:, 12 * C:WCOLS])

            # HAM warm-up: dep-free junk matmuls lift the PE clock gate
            # while the head DMAs stream in.
            wmsrc = wpool.tile([C, 512], f16)
            nc.vector.memset(wmsrc[:, :], 0.0)
            wmps = pspool.tile([C, FD], f32, tag="ps0")
            for _ in range(N_WARMUP):
                nc.tensor.matmul(wmps[:, :], lhsT=wmsrc[:, 0:C],
                                 rhs=wmsrc[:, 0:512], start=True, stop=True,
                                 skip_group_check=True)

            # input chunks: one group's 4 tile-blocks each (contiguous)
            def emit_chunk(g):
                lo, hi = g * GPT * TBLK, (g + 1) * GPT * TBLK
                nc.sync.dma_start(out=xfull[:, lo:hi], in_=xb[:, lo:hi])

            for g in range(4):
                emit_chunk(g)

            for g in range(NG):
                if g < NG - 4:
                    emit_chunk(g + 4)
                ps = [pspool.tile([C, FD], f32, name=f"psu{u}", tag=f"ps{u}")
                      for u in range(4)]
                for u in (1, 2, 0, 3):
                    for dw in range(3):
                        base = g * GPT * TBLK + ROWOFF[u] + dw
                        rhs = bass.AP(xfull.tensor, xfull.offset + base,
                                      [xfull.ap[0], [TBLK, GPT], [1, W]])
                        nc.tensor.matmul(
                            ps[u][:, :],
                            lhsT=wtile[:, (u * 3 + dw) * C:(u * 3 + dw + 1) * C],
                            rhs=rhs,
                            start=(dw == 0),
                            stop=(dw == 2),
                        )
                # Engine constraints: GPSIMD cannot touch PSUM; DVE may read
                # only ONE PSUM operand per op; ACT is 2.3x errata-slow, so
                # keep it off the data path entirely.  DVE extracts s1/s2
                # from PSUM and does the PSUM-touching fused ops; GPSIMD
                # takes the two SBUF-only adds.
                s1 = tpool.tile([C, FD], f32)
                s2 = tpool.tile([C, FD], f32)
                nc.vector.tensor_scalar_add(out=s1[:, :], in0=ps[1][:, :],
                                            scalar1=btile[:, 0:1])
                nc.vector.tensor_copy(out=s2[:, :], in_=ps[2][:, :])
                # even = (m0 + s1) + s2 ;  odd = (s1 - s2) - m3
                tmp_e = tpool.tile([C, FD], f32)
                obuf_e = opool.tile([C, FD], f16)
                nc.vector.scalar_tensor_tensor(
                    out=tmp_e[:, :], in0=ps[0][:, :], scalar=0.0,
                    in1=s1[:, :], op0=ADD, op1=ADD)
                nc.gpsimd.tensor_add(obuf_e[:, :], tmp_e[:, :], s2[:, :])
                tmp_o = tpool.tile([C, FD], f32)
                obuf_o = opool.tile([C, FD], f16)
                nc.gpsimd.tensor_sub(tmp_o[:, :], s1[:, :], s2[:, :])
                nc.vector.scalar_tensor_tensor(
                    out=obuf_o[:, :], in0=ps[3][:, :], scalar=-1.0,
                    in1=tmp_o[:, :], op0=MUL, op1=ADD)
                nc.scalar.dma_start(out=oute[:, g * GPT:(g + 1) * GPT, :],
                                    in_=obuf_e[:, :])
                nc.scalar.dma_start(out=outo[:, g * GPT:(g + 1) * GPT, :],
                                    in_=obuf_o[:, :])

    nc.compile()
    return nc


def _get_nc():
    global _CACHED_NC
    if _CACHED_NC is None:
        _CACHED_NC = _build_nc()
    return _CACHED_NC


def _prep_weights(Wfull: np.ndarray, b: np.ndarray):
    # V[ci*8+k, dh, dw, co*8+m] = sum_j S[m,j,k] * W[j, co, ci, dh, dw]
    V = np.einsum("mjk,jcihw->ikhwcm", _S.astype(np.float64),
                  np.asarray(Wfull).astype(np.float64)).reshape(C, 3, 3, C)
    G = [V[:, 0], (V[:, 0] + V[:, 1] + V[:, 2]) / 2,
         (V[:, 0] - V[:, 1] + V[:, 2]) / 2, V[:, 2]]   # each [ic, dw, oc]
    wf = np.empty((C, WCOLS), dtype=np.float16)
    for u in range(4):
        for dw in range(3):
            wf[:, (u * 3 + dw) * C:(u * 3 + dw + 1) * C] = G[u][:, dw, :]
    bias = np.einsum("mjk,jc->cm", _S.astype(np.float64),
                     np.asarray(b).astype(np.float64)).reshape(C)
    wf[:, 12 * C] = bias.astype(np.float16)
    return np.ascontiguousarray(wf)


def _prep_inputs(x: np.ndarray) -> np.ndarray:
    # [B, C, H, W] -> Winograd row-transformed flat planes [B, C, FLAT]
    nB = x.shape[0]
    pr = np.zeros((nB, C, H + 2, W), dtype=np.float32)
    pr[:, :, 1:-1, :] = x
    xt = np.zeros((nB, C, NT, 4, PWR), dtype=np.float16)
    xt[:, :, :, 0, 1:W + 1] = pr[:, :, 0:2 * NT:2] - pr[:, :, 2:2 * NT + 2:2]
    xt[:, :, :, 1, 1:W + 1] = pr[:, :, 1:2 * NT + 1:2] - pr[:, :, 3:2 * NT + 3:2]
    xt[:, :, :, 2, 1:W + 1] = pr[:, :, 1:2 * NT + 1:2] + pr[:, :, 2:2 * NT + 2:2]
    xt[:, :, :, 3, 1:W + 1] = pr[:, :, 2:2 * NT + 2:2] - pr[:, :, 1:2 * NT + 1:2]
    return xt.reshape(nB, C, FLAT)


def kernel(x: np.ndarray, W: np.ndarray, b: np.ndarray) -> np.ndarray:
    from concourse.bass_utils import run_bass_kernel_spmd

    xt = _prep_inputs(np.ascontiguousarray(x, dtype=np.float32))
    wf = _prep_weights(W, b)

    nc = _get_nc()
    in_maps = [{"xb": xt[c], "wf": wf} for c in range(N_CORES)]
    res = run_bass_kernel_spmd(nc, in_maps, core_ids=list(range(N_CORES)))
    out = np.empty((N_CORES, C, H, 128), dtype=np.float32)
    for c in range(N_CORES):
        out[c, :, 0::2, :] = res.results[c]["oute"].astype(np.float32)
        out[c, :, 1::2, :] = res.results[c]["outo"].astype(np.float32)
    return out


# revision 16
# speedup vs baseline: 1.1047x; 1.0059x over previous
"""GA3 Conv2d kernel for 8 Trainium2 NeuronCores — Winograd F(2,3) along H.

Math: the sign-combination einsum folds into the conv weights, making the
module ONE dense 3x3 conv with Cin=Cout=128 on [B, 128, 128, 128] (see
_prep math below).  We shard data-parallel over B (1 image per core).

The direct 9-tap matmul formulation is PE-bound: 9 taps x 512 cols x 32
groups = 147k PE cycles, ~70 us at the power-throttled ~2.1 GHz clock.
Winograd F(2,3) applied along H cuts PE work to 2/3: for each pair of
output rows (2c, 2c+1) and input rows d0..d3 = padded rows 2c..2c+3,
    m0 = conv_w(t0) @ G0,  t0 = d0 - d2,   G0 = W[dh=0]
    m1 = conv_w(t1) @ G1,  t1 = d1 + d2,   G1 = (W0+W1+W2)/2
    m2 = conv_w(t2) @ G2,  t2 = d2 - d1,   G2 = (W0-W1+W2)/2
    m3 = conv_w(t3) @ G3,  t3 = d1 - d3,   G3 = W[dh=2]
    even row = m0 + m1 + m2 + bias ;  odd row = m1 - m2 - m3 + bias
where conv_w is the remaining direct 3-tap conv along W (dw = 0..2).
The row transforms t0..t3 are LINEAR and cheap -> computed on HOST and
shipped as fp16 planes (2x input bytes, still under the PE roofline).
The output combine runs on DVE (even rows) and GPSIMD (odd rows) as two
fused scalar_tensor_tensor passes each, overlapping the PE stream; the
bias rides the STT per-partition scalar operand for free.

Layout: per tile c (c = 0..63) the host packs 4 width-padded rows
[t0 | t3 | t1 | t2], each [1 pad][128 data][1 pad] = 130 elems, so every
matmul rhs is a [520-stride x 4 tiles][1 x 128] AP and all loads are
contiguous chunks.  Even/odd output rows go to separate DRAM planes in
fp16; the host interleaves and upcasts (measured rel err ~5e-4).
"""

import numpy as np

_TERMS = [
    [(0, 0, 1), (1, 1, 1), (2, 2, 1), (3, 3, 1), (4, 4, -1), (5, 5, -1), (6, 6, -1), (7, 7, -1)],
    [(1, 0, 1), (0, 1, 1), (2, 4, 1), (4, 2, -1), (3, 6, 1), (6, 3, -1), (5, 7, -1), (7, 5, -1)],
    [(2, 0, 1), (0, 2, 1), (1, 4, -1), (4, 1, 1), (3, 5, 1), (5, 3, -1), (6, 7, 1), (7, 6, 1)],
    [(3, 0, 1), (0, 3, 1), (1, 6, -1), (6, 1, 1), (2, 5, -1), (5, 2, 1), (4, 7, -1), (7, 4, -1)],
    [(4, 0, 1), (0, 4, 1), (2, 1, 1), (1, 2, -1), (3, 7, 1), (7, 3, 1), (6, 5, 1), (5, 6, -1)],
    [(5, 0, 1), (0, 5, 1), (3, 2, 1), (2, 3, -1), (1, 7, 1), (7, 1, 1), (4, 6, 1), (6, 4, -1)],
    [(6, 0, 1), (0, 6, 1), (3, 1, 1), (1, 3, -1), (2, 7, -1), (7, 2, -1), (5, 4, 1), (4, 5, -1)],
    [(7, 0, 1), (0, 7, 1), (5, 1, 1), (1, 5, 1), (6, 2, -1), (2, 6, -1), (4, 3, 1), (3, 4, 1)],
]
_S = np.zeros((8, 8, 8), dtype=np.float32)
for _m, _terms in enumerate(_TERMS):
    for _j, _k, _s in _terms:
        _S[_m, _j, _k] = _s

B, CIN, COUT, H, W = 8, 16, 16, 128, 128
C = 8 * CIN          # 128 interleaved channels
N_CORES = 8
NT = H // 2          # 64 row-tiles (one per output row pair)
PWR = W + 2          # padded row: [pad][128][pad]
TBLK = 4 * PWR       # per-tile block: rows [t0, t3, t1, t2]
FLAT = NT * TBLK     # flat elems/partition
ROWOFF = [0, 2 * PWR, 3 * PWR, PWR]   # plane offset of t_u within a block
GPT = 4              # tiles per PSUM group (4 tiles = 4 even + 4 odd rows)
NG = NT // GPT       # 16 groups
FD = GPT * W         # 512 matmul free dim / combine span
WCOLS = 12 * C + 1   # 12 transformed weight mats + bias column
N_WARMUP = 11        # HAM warm-up matmuls during the head DMAs

_CACHED_NC = None


def _build_nc():
    import concourse.bass as bass
    import concourse.mybir as mybir
    import concourse.tile as tile
    from concourse import bacc

    f32 = mybir.dt.float32
    f16 = mybir.dt.float16
    ADD = mybir.AluOpType.add
    SUB = mybir.AluOpType.subtract
    MUL = mybir.AluOpType.mult

    nc = bacc.Bacc("TRN2", target_bir_lowering=False, debug=False,
                   enable_asserts=False)

    xb = nc.dram_tensor("xb", [C, FLAT], f16, kind="ExternalInput").ap()
    wf = nc.dram_tensor("wf", [C, WCOLS], f16, kind="ExternalInput").ap()
    oute = nc.dram_tensor("oute", [C, NT, W], f16, kind="ExternalOutput").ap()
    outo = nc.dram_tensor("outo", [C, NT, W], f16, kind="ExternalOutput").ap()

    with tile.TileContext(nc) as tc:
        with (
            tc.tile_pool(name="wpool", bufs=1) as wpool,
            tc.tile_pool(name="xpool", bufs=1) as xpool,
            tc.tile_pool(name="pspool", bufs=2, space="PSUM") as pspool,
            tc.tile_pool(name="tpool", bufs=8) as tpool,
            tc.tile_pool(name="opool", bufs=6) as opool,
        ):
            xfull = xpool.tile([C, FLAT], f16)
            wtile = wpool.tile([C, WCOLS], f16)

            # Loads ride the SP ring in FIFO order; stores get the ACT ring.
            nc.sync.dma_start(out=wtile[:, :], in_=wf[:, :])
            btile = wpool.tile([C, 1], f32)
            nc.vector.tensor_copy(out=btile[:, :], in_=wtile[:, 12 * C:WCOLS])

            # HAM warm-up: dep-free junk matmuls lift the PE clock gate
            # while the head DMAs stream in.
            wmsrc = wpool.tile([C, 512], f16)
            nc.vector.memset(wmsrc[:, :], 0.0)
            wmps = pspool.tile([C, FD], f32, tag="ps0")
            for _ in range(N_WARMUP):
                nc.tensor.matmul(wmps[:, :], lhsT=wmsrc[:, 0:C],
                                 rhs=wmsrc[:, 0:512], start=True, stop=True,
                                 skip_group_check=True)

            # input chunks: one group's 4 tile-blocks each (contiguous)
            def emit_chunk(g):
                lo, hi = g * GPT * TBLK, (g + 1) * GPT * TBLK
                nc.sync.dma_start(out=xfull[:, lo:hi], in_=xb[:, lo:hi])

            for g in range(4):
                emit_chunk(g)

            for g in range(NG):
                if g < NG - 4:
                    emit_chunk(g + 4)
                ps = [pspool.tile([C, FD], f32, name=f"psu{u}", tag=f"ps{u}")
                      for u in range(4)]
                for u in (1, 2, 0, 3):
                    for dw in range(3):
                        base = g * GPT * TBLK + ROWOFF[u] + dw
                        rhs = bass.AP(xfull.tensor, xfull.offset + base,
                                      [xfull.ap[0], [TBLK, GPT], [1, W]])
                        nc.tensor.matmul(
                            ps[u][:, :],
                            lhsT=wtile[:, (u * 3 + dw) * C:(u * 3 + dw + 1) * C],
                            rhs=rhs,
                            start=(dw == 0),
                            stop=(dw == 2),
                        )
                # Engine constraints: GPSIMD cannot touch PSUM; DVE may read
                # only ONE PSUM operand per op; ACT is 2.3x errata-slow, so
                # keep it off the data path entirely.  DVE extracts s1/s2
                # from PSUM and does the PSUM-touching fused ops; GPSIMD
                # takes the two SBUF-only adds.
                # Emission order matters: every engine drains its queue in
                # order, so instructions are emitted in dependency-readiness
                # order to avoid cross-engine head-of-line blocking.
                s1 = tpool.tile([C, FD], f32)
                s2 = tpool.tile([C, FD], f32)
                tmp_e = tpool.tile([C, FD], f32)
                tmp_o = tpool.tile([C, FD], f32)
                obuf_e = opool.tile([C, FD], f16)
                obuf_o = opool.tile([C, FD], f16)
                nc.vector.tensor_scalar_add(out=s1[:, :], in0=ps[1][:, :],
                                            scalar1=btile[:, 0:1])
                nc.vector.tensor_copy(out=s2[:, :], in_=ps[2][:, :])
                nc.gpsimd.tensor_sub(tmp_o[:, :], s1[:, :], s2[:, :])
                # even = (m0 + s1) + s2 ;  odd = (s1 - s2) - m3
                nc.vector.scalar_tensor_tensor(
                    out=tmp_e[:, :], in0=ps[0][:, :], scalar=0.0,
                    in1=s1[:, :], op0=ADD, op1=ADD)
                nc.vector.scalar_tensor_tensor(
                    out=obuf_o[:, :], in0=ps[3][:, :], scalar=-1.0,
                    in1=tmp_o[:, :], op0=MUL, op1=ADD)
                nc.gpsimd.tensor_add(obuf_e[:, :], tmp_e[:, :], s2[:, :])
                nc.scalar.dma_start(out=oute[:, g * GPT:(g + 1) * GPT, :],
                                    in_=obuf_e[:, :])
                nc.scalar.dma_start(out=outo[:, g * GPT:(g + 1) * GPT, :],
                                    in_=obuf_o[:, :])

    nc.compile()
    return nc


def _get_nc():
    global _CACHED_NC
    if _CACHED_NC is None:
        _CACHED_NC = _build_nc()
    return _CACHED_NC


def _prep_weights(Wfull: np.ndarray, b: np.ndarray):
    # V[ci*8+k, dh, dw, co*8+m] = sum_j S[m,j,k] * W[j, co, ci, dh, dw]
    V = np.einsum("mjk,jcihw->ikhwcm", _S.astype(np.float64),
                  np.asarray(Wfull).astype(np.float64)).reshape(C, 3, 3, C)
    G = [V[:, 0], (V[:, 0] + V[:, 1] + V[:, 2]) / 2,
         (V[:, 0] - V[:, 1] + V[:, 2]) / 2, V[:, 2]]   # each [ic, dw, oc]
    wf = np.empty((C, WCOLS), dtype=np.float16)
    for u in range(4):
        for dw in range(3):
            wf[:, (u * 3 + dw) * C:(u * 3 + dw + 1) * C] = G[u][:, dw, :]
    bias = np.einsum("mjk,jc->cm", _S.astype(np.float64),
                     np.asarray(b).astype(np.float64)).reshape(C)
    wf[:, 12 * C] = bias.astype(np.float16)
    return np.ascontiguousarray(wf)


def _prep_inputs(x: np.ndarray) -> np.ndarray:
    # [B, C, H, W] -> Winograd row-transformed flat planes [B, C, FLAT]
    nB = x.shape[0]
    pr = np.zeros((nB, C, H + 2, W), dtype=np.float32)
    pr[:, :, 1:-1, :] = x
    xt = np.zeros((nB, C, NT, 4, PWR), dtype=np.float16)
    xt[:, :, :, 0, 1:W + 1] = pr[:, :, 0:2 * NT:2] - pr[:, :, 2:2 * NT + 2:2]
    xt[:, :, :, 1, 1:W + 1] = pr[:, :, 1:2 * NT + 1:2] - pr[:, :, 3:2 * NT + 3:2]
    xt[:, :, :, 2, 1:W + 1] = pr[:, :, 1:2 * NT + 1:2] + pr[:, :, 2:2 * NT + 2:2]
    xt[:, :, :, 3, 1:W + 1] = pr[:, :, 2:2 * NT + 2:2] - pr[:, :, 1:2 * NT + 1:2]
    return xt.reshape(nB, C, FLAT)


def kernel(x: np.ndarray, W: np.ndarray, b: np.ndarray) -> np.ndarray:
    from concourse.bass_utils import run_bass_kernel_spmd

    xt = _prep_inputs(np.ascontiguousarray(x, dtype=np.float32))
    wf = _prep_weights(W, b)

    nc = _get_nc()
    in_maps = [{"xb": xt[c], "wf": wf} for c in range(N_CORES)]
    res = run_bass_kernel_spmd(nc, in_maps, core_ids=list(range(N_CORES)))
    out = np.empty((N_CORES, C, H, 128), dtype=np.float32)
    for c in range(N_CORES):
        out[c, :, 0::2, :] = res.results[c]["oute"].astype(np.float32)
        out[c, :, 1::2, :] = res.results[c]["outo"].astype(np.float32)
    return out
